# revision 3
# baseline (speedup 1.0000x reference)
"""CurveGrouping kernel for 8 NeuronCores (Trainium2, axon backend).

Strategy (per the data-parallel sharding hint): shard x/idx on the batch dim
(bn=8 -> one sample per core), replicate the tiny MLP/BN params. The two
training-mode BatchNorms need cross-core statistics every step; those are the
only cross-core dependencies.

The two per-step BN stat exchanges are fused into ONE all_gather of per-curve
scalars: with a = w2.cur(t-1), b = w2.pre(t-1) and z = mom_w @ [cur; pre],
every core can reconstruct ALL cores' agent-BN sums from
{z0, z1, a, b, S1sum, S1sq} after locally applying the mom BN (att weights),
because w2.pre(t) = att0*a + att1*b and the agent logits are
raw = s1[pick] + w2.pre(t).

neuronx-cc cannot compile variadic reduces (argmax/top_k), so:
  - argmax is done as max + "first index attaining the max" (two reduces)
  - the top-256 starting points are computed host-side (a trivial 2M-MAC
    matvec + argsort) and passed in as an extra sharded input
"""

import numpy as np
import jax
import jax.numpy as jnp
from jax import lax
from jax.sharding import Mesh, PartitionSpec as P
from jax.experimental.shard_map import shard_map

BN_, C, N, K = 8, 64, 16384, 32
CURVE_NUM, CURVE_LEN = 256, 32
EPS_BN = 1e-5
AXIS = "b"
BIGI = np.float32(1e9)


def _argmax_last(v):
    """First index of the max along the last axis, via single-operand reduces."""
    vmax = jnp.max(v, axis=-1, keepdims=True)
    ar = lax.broadcasted_iota(jnp.float32, v.shape, v.ndim - 1)
    cand = jnp.where(v >= vmax, ar, BIGI)
    return jnp.min(cand, axis=-1).astype(jnp.int32)


def _onehot_last(sel, n):
    ar = lax.broadcasted_iota(jnp.int32, sel.shape + (n,), sel.ndim)
    return (sel[..., None] == ar).astype(jnp.float32)


def _local_forward(x, idx, start, att_w, agent_w, agent_g, agent_b,
                   mom_w, mom_g, mom_b):
    # x: [1, C, N]  idx: [1, N, K]  start: [1, cn] -- this core's sample
    _, c, n = x.shape
    k = idx.shape[-1]
    cn = CURVE_NUM
    nb_dev = BN_

    w1 = agent_w[:c]
    w2 = agent_w[c:]
    mwc = mom_w[:, :c]
    mwp = mom_w[:, c:]

    # ---- Phase A: attention gate, weighted features, per-point precomputes
    x0 = x[0]                                                  # [C, N]
    x_att = jax.nn.sigmoid(jnp.einsum("cn,c->n", x0, att_w))   # [N]
    xw_t = (x0 * x_att[None, :]).T                             # [N, C]
    s1 = xw_t @ w1                                             # [N]

    adj = idx[0]                                               # [N, K]

    gcnt_a = np.float32(nb_dev * cn * k)

    def agent_bn(raw, gsum, gsq):
        mean = gsum / gcnt_a
        var = gsq / gcnt_a - mean * mean
        return (raw - mean) / jnp.sqrt(var + EPS_BN) * agent_g[0] + agent_b[0]

    def sel_gather(pv, pick_idx, sel):
        oh = _onehot_last(sel, k)                              # [cn, K]
        cur = jnp.einsum("mkc,mk->mc", pv, oh)                 # [cn, C]
        fcur = jnp.sum(pick_idx * oh.astype(jnp.int32), axis=1)
        return cur, fcur

    # ---- step 0 (no momentum gate, no crossover suppression)
    fcur = start[0]                                            # [cn]
    sp = xw_t[fcur]                                            # [cn, C]
    pick_idx = adj[fcur]                                       # [cn, K]
    pv = xw_t[pick_idx]                                        # [cn, K, C]
    s1p = s1[pick_idx]                                         # [cn, K]
    raw = s1p + (sp @ w2)[:, None]                             # [cn, K]

    loc = jnp.stack([jnp.sum(raw), jnp.sum(raw * raw)])
    glob = lax.psum(loc, AXIS)
    logits = agent_bn(raw, glob[0], glob[1])
    sel = _argmax_last(logits)
    cur, fcur = sel_gather(pv, pick_idx, sel)
    pre = sp
    first = cur.T                                              # [C, cn]

    def step(carry, _):
        pre, cur, fcur = carry
        z = cur @ mwc.T + pre @ mwp.T                          # [cn, 2]
        a_s = cur @ w2
        b_s = pre @ w2
        pick_idx = adj[fcur]                                   # [cn, K]
        s1p = s1[pick_idx]
        s1sum = jnp.sum(s1p, axis=1)
        s1sq = jnp.sum(s1p * s1p, axis=1)

        payload = jnp.stack([z[:, 0], z[:, 1], a_s, b_s, s1sum, s1sq])
        allp = lax.all_gather(payload, AXIS)                   # [nb, 6, cn]

        z0g, z1g = allp[:, 0], allp[:, 1]
        m0 = jnp.mean(z0g)
        v0 = jnp.mean(z0g * z0g) - m0 * m0
        m1 = jnp.mean(z1g)
        v1 = jnp.mean(z1g * z1g) - m1 * m1
        r0 = 1.0 / jnp.sqrt(v0 + EPS_BN)
        r1 = 1.0 / jnp.sqrt(v1 + EPS_BN)

        def gate_weights(bn0, bn1):
            # reference applies softmax over the 2 channels then does a RAW
            # reshape [2, cn] -> [cn, 2] (torch .view semantics): curve m<128
            # gets (ch0[2m], ch0[2m+1]); m>=128 gets (ch1[2m-256], ch1[2m-255])
            sm0 = jax.nn.sigmoid(bn0 - bn1)
            p0 = sm0.reshape(sm0.shape[:-1] + (cn // 2, 2))
            p1 = (1.0 - sm0).reshape(sm0.shape[:-1] + (cn // 2, 2))
            wc = jnp.concatenate([p0[..., 0], p1[..., 0]], axis=-1)
            wp = jnp.concatenate([p0[..., 1], p1[..., 1]], axis=-1)
            return wc, wp

        bn0g = (z0g - m0) * r0 * mom_g[0] + mom_b[0]           # [nb, cn]
        bn1g = (z1g - m1) * r1 * mom_g[1] + mom_b[1]
        wcg, wpg = gate_weights(bn0g, bn1g)
        t2g = wcg * allp[:, 2] + wpg * allp[:, 3]
        gsum = jnp.sum(allp[:, 4]) + k * jnp.sum(t2g)
        gsq = (jnp.sum(allp[:, 5]) + 2.0 * jnp.sum(t2g * allp[:, 4])
               + k * jnp.sum(t2g * t2g))

        # local mom gate + feature blend
        bn0 = (z[:, 0] - m0) * r0 * mom_g[0] + mom_b[0]
        bn1 = (z[:, 1] - m1) * r1 * mom_g[1] + mom_b[1]
        att0, att1 = gate_weights(bn0, bn1)
        pre_new = att0[:, None] * cur + att1[:, None] * pre

        pv = xw_t[pick_idx]                                    # [cn, K, C]
        raw = s1p + (att0 * a_s + att1 * b_s)[:, None]
        logits = agent_bn(raw, gsum, gsq)

        # crossover suppression
        a_vec = cur - pre_new
        nbv = pv - cur[:, None, :]
        dot = jnp.einsum("mc,mkc->mk", a_vec, nbv)
        div = jnp.clip(
            jnp.sqrt(jnp.sum(a_vec * a_vec, axis=1))[:, None]
            * jnp.sqrt(jnp.sum(nbv * nbv, axis=2)),
            1e-8, None)
        d = jnp.clip(1.0 + dot / div, 0.0, 1.0)
        logits = logits * d

        sel = _argmax_last(logits)
        cur_new, fcur_new = sel_gather(pv, pick_idx, sel)
        return (pre_new, cur_new, fcur_new), cur_new.T

    carry = (pre, cur, fcur)
    _, rest = lax.scan(step, carry, None, length=CURVE_LEN - 1)
    curves = jnp.concatenate([first[None], rest], axis=0)      # [L, C, cn]
    return jnp.transpose(curves, (1, 2, 0))[None]              # [1, C, cn, L]


_JITTED = None


def _build():
    devs = jax.devices()[:8]
    mesh = Mesh(np.asarray(devs), (AXIS,))
    fn = shard_map(
        _local_forward,
        mesh=mesh,
        in_specs=(P(AXIS), P(AXIS), P(AXIS), P(), P(), P(), P(), P(), P(), P()),
        out_specs=P(AXIS),
        check_rep=False,
    )
    return jax.jit(fn)


def _host_topk_starts(x, att_w):
    # top-256 starting points per sample, descending value, ties -> lower idx
    # (matches jax.lax.top_k). fp32 matvec like the reference.
    logits = np.einsum("bcn,c->bn", x.astype(np.float32),
                       att_w.astype(np.float32), optimize=True)
    # sigmoid is monotonic; rank directly on logits
    order = np.argsort(-logits, axis=1, kind="stable")[:, :CURVE_NUM]
    return order.astype(np.int32)


def kernel(x, xyz, idx, att_w, agent_w, agent_g, agent_b, mom_w, mom_g, mom_b):
    global _JITTED
    if _JITTED is None:
        _JITTED = _build()
    start = _host_topk_starts(np.asarray(x), np.asarray(att_w))
    out = _JITTED(
        jnp.asarray(x),
        jnp.asarray(idx, dtype=jnp.int32),
        jnp.asarray(start),
        jnp.asarray(att_w),
        jnp.asarray(agent_w),
        jnp.asarray(agent_g),
        jnp.asarray(agent_b),
        jnp.asarray(mom_w),
        jnp.asarray(mom_g),
        jnp.asarray(mom_b),
    )
    return np.asarray(jax.device_get(out)).astype(np.float32)
